# revision 25
# baseline (speedup 1.0000x reference)
"""Trainium2 Bass kernel for the Koopman control-model chain (v3).

Computes, for fixed-size inputs L[4096,4096], R[2048,2048], B[2048,256]:
    M   = L @ L.T            (blocks M11, M21, M22 only)
    F   = M21, P = M22
    E   = (M11 + M22)/2 + (R - R.T)/2
    Acl = solve(E, F)        (block LU without pivoting, Newton-inverted
                              128x128 diagonal blocks)
    A   = (I - B @ (B.T @ P)) @ Acl

Distribution: 8 NeuronCores, column-sharded (each core owns a 256-column
slice of every 2048-wide intermediate).  The Gram phase and the triangular
substitutions are column-parallel; E's factorization is replicated on all
cores after an AllGather of S = (M11+M22)/2 + skew.

v3 structure:
  - Gram is split S-first: the S = 0.5(M11+M22)+skew row-blocks are
    computed first (two 32-matmul chains per row-block), so the four
    AllGather quarters kick ~2x earlier; the F = M21 pass runs after
    and doubles as PE filler under the early Newton chains.
  - Everything downstream of the Gram runs in bf16 (fp32 PSUM):
    FWL weight loads + 2x matmul streaming.
  - S is AllGathered in bf16 and DMAed straight into the factorization
    tiles es[s][i]; a dummy 1-element AllGather at program start absorbs
    the collective entry barrier.
  - The factorization is emitted on a (column x quarter) frontier: each
    column's pivot update + Newton chain is emitted as soon as its
    pivot rows' quarter is available, with the bulk row work (panels,
    odd terms, strip-entry updates, U-row updates) run inline up to the
    arrived-row frontier and deferred per-quarter otherwise.  The
    generator is pumped from inside the S-pass loop so Newton chains
    overlap the Gram and the AllGathers.
  - The per-strip left-looking U-row cascade is spread across columns
    (U-row k emitted right after panels(k-1)).
  - Newton tracks X and X^T in one [128,256] tile pair with a single
    fused PSUM->SBUF cast per iteration; panel copy-backs ride the
    otherwise idle Scalar engine (except the next-pivot pair).
  - Back-substitution results are additionally stored in fp32 and the
    final output subtraction uses them (removes X's bf16 rounding from
    the output).
"""

import ml_dtypes
import numpy as np

import concourse.bass as bass  # noqa: F401  (registers engines)
import concourse.mybir as mybir
import concourse.tile as tile
from concourse import bacc
from concourse.bass_utils import run_bass_kernel_spmd

F32 = mybir.dt.float32
BF16 = mybir.dt.bfloat16
P = 128

LAST_EXEC_NS = None


class Cfg:
    def __init__(self, d=2048, v=256, ncores=8, newton0=26, newton=12,
                 warm=160):
        self.d = d                    # dim_K
        self.v = v                    # B's column count
        self.ncores = ncores
        self.kdim = 2 * d             # Gram contraction length (rows of L.T)
        self.cw = d // ncores         # per-core column slice
        assert self.cw == 256, "strip width must equal per-core slice (256)"
        self.nb = d // P              # 128-blocks along d
        self.kt = self.kdim // P      # k-tiles in the Gram contraction
        self.ns = self.nb // 2        # 256-wide strips
        self.nv = v // P              # 128-blocks along v
        self.newton = [newton0] + [newton] * (self.nb - 1)
        self.warm = warm


def build_program(cfg: Cfg):
    d, v, cw, nb, kt, ns, nv = (cfg.d, cfg.v, cfg.cw, cfg.nb, cfg.kt,
                                cfg.ns, cfg.nv)
    nc = bacc.Bacc("TRN2", target_bir_lowering=False, debug=False,
                   num_devices=cfg.ncores)

    lt_in = nc.dram_tensor("lt", [cfg.kdim, cfg.kdim], BF16,
                           kind="ExternalInput").ap()
    rhs_in = nc.dram_tensor("rhs", [P, kt * 2 * cw], BF16,
                            kind="ExternalInput").ap()
    b_in = nc.dram_tensor("b", [P, nb * v], BF16, kind="ExternalInput").ap()
    bt_in = nc.dram_tensor("bt", [P, nv * d], BF16, kind="ExternalInput").ap()
    sk_in = nc.dram_tensor("sk", [P, nb * cw], BF16, kind="ExternalInput").ap()
    iden_in = nc.dram_tensor("iden", [P, P], BF16, kind="ExternalInput").ap()
    iden2_in = nc.dram_tensor("iden2", [P, P], F32, kind="ExternalInput").ap()
    ones_in = nc.dram_tensor("onesv", [P, 1], F32, kind="ExternalInput").ap()
    onesr_in = nc.dram_tensor("onesr", [1, P], F32, kind="ExternalInput").ap()
    a_out = nc.dram_tensor("a_out", [P, nb * cw], F32,
                           kind="ExternalOutput").ap()

    rg = [list(range(cfg.ncores))]
    sub = mybir.AluOpType.subtract
    add = mybir.AluOpType.add
    mult = mybir.AluOpType.mult
    CopyFn = mybir.ActivationFunctionType.Copy
    QB = nb // 4  # row-blocks per S AllGather quarter

    with tile.TileContext(nc) as tc:
        with (
            tc.tile_pool(name="const", bufs=1) as const,
            tc.tile_pool(name="flong", bufs=1) as flong,
            tc.tile_pool(name="epool", bufs=1) as epool,
            tc.tile_pool(name="nwt", bufs=2) as nwt,
            tc.tile_pool(name="stg", bufs=4) as stg,
            tc.tile_pool(name="psF", bufs=4, space="PSUM") as psF,
            tc.tile_pool(name="dram", bufs=1, space="DRAM") as dram,
        ):
            iden = const.tile([P, P], BF16, tag="iden")
            iden2 = const.tile([P, P], F32, tag="iden2")
            ones_c = const.tile([P, 1], F32, tag="ones_c")
            ones_r = const.tile([1, P], F32, tag="ones_r")
            nc.sync.dma_start(iden[:], iden_in[:])
            nc.sync.dma_start(iden2[:], iden2_in[:])
            nc.sync.dma_start(ones_c[:], ones_in[:])
            nc.sync.dma_start(ones_r[:], onesr_in[:])

            # F-tiles (later Y, then X): one [128, cw] bf16 tile per row-block
            fY = [flong.tile([P, cw], BF16, tag=f"fY{i}", name=f"fY{i}")
                  for i in range(nb)]

            # Factorization storage, replicated 2048x2048 in bf16:
            # es[s][i] = E-block-row i of column strip s ([128, 256]).
            es = [[epool.tile([P, 2 * P], BF16, tag=f"e{s}_{i}",
                              name=f"e{s}_{i}")
                   for i in range(nb)] for s in range(ns)]
            # w2[j] = [ W | W^T ] for D_j^-1
            w2 = [epool.tile([P, 2 * P], BF16, tag=f"w{j}", name=f"w{j}")
                  for j in range(nb)]

            def wN(j):
                return w2[j][:, 0:P]

            def wT(j):
                return w2[j][:, P:2 * P]

            # DRAM staging for the S AllGather quarters, t-major per core so
            # the gathered rows land as contiguous [128, 256] blocks that DMA
            # straight into es tiles.
            sag_b = [dram.tile([P * QB, cw], BF16, name=f"sagb{q}")
                     for q in range(4)]
            sagg = [dram.tile([cfg.ncores * P * QB, cw], BF16,
                              addr_space="Shared", name=f"sagg{q}")
                    for q in range(4)]
            u1_b = dram.tile([cw, v], BF16)
            u1ag = dram.tile([cfg.ncores * cw, v], BF16, addr_space="Shared")
            sink = dram.tile([1, 1], F32)
            sink2 = dram.tile([1, 2], BF16)
            dummy_b = dram.tile([1, 2], BF16, name="dummyb")
            dummy_g = dram.tile([cfg.ncores, 2], BF16, addr_space="Shared",
                                name="dummyg")

            # ---------------- factorization helpers ----------------
            def etile(i, k):
                """[128,128] slice of E-storage at block (i, k)."""
                return es[k // 2][i][:, (k % 2) * P:(k % 2 + 1) * P]

            def pe_transpose(src_ap, tag="tps"):
                """128x128 bf16 transpose via PE; returns an SBUF tile."""
                pst = psF.tile([P, P], BF16, tag="tpp", bufs=2)
                nc.tensor.transpose(pst[:], src_ap, iden[:])
                out = stg.tile([P, P], BF16, tag=tag, bufs=2)
                nc.vector.tensor_copy(out[:], pst[:])
                return out

            def newton(j, d_n):
                """Invert D_j; writes w2[j] = [W | W^T]."""
                # dT lives across the whole iteration chain: own tag so
                # other transposes can't recycle its slot underneath it.
                dT = pe_transpose(d_n, tag="dT")
                # alpha = 1 / sum(D*D)
                sq = stg.tile([P, P], F32, tag="sq", bufs=2)
                nc.vector.tensor_tensor(sq[:], d_n, d_n, op=mult)
                rowsum = stg.tile([P, 1], F32, tag="rsum")
                nc.vector.tensor_reduce(rowsum[:], sq[:],
                                        axis=mybir.AxisListType.X, op=add)
                pss = psF.tile([P, 2 * P], F32, tag="mm")
                nc.tensor.matmul(pss[0:1, 0:1], rowsum[:], ones_c[:],
                                 start=True, stop=True)
                alph = stg.tile([1, 1], F32, tag="alph")
                nc.vector.reciprocal(alph[:], pss[0:1, 0:1])
                psb = psF.tile([P, 2 * P], F32, tag="mm")
                nc.tensor.matmul(psb[:, 0:1], ones_r[:], alph[:],
                                 start=True, stop=True)
                ab = stg.tile([P, 1], F32, tag="ab")
                nc.vector.tensor_copy(ab[:], psb[:, 0:1])
                # xy = [X | X^T]; X0 = alpha D^T
                xy = nwt.tile([P, 2 * P], BF16, tag="xy")
                nc.vector.tensor_scalar_mul(xy[:, 0:P], dT[:], ab[:])
                nc.vector.tensor_scalar_mul(xy[:, P:2 * P], d_n, ab[:])
                # Newton-Schulz in 2I-DX form:
                #   Z2 = 2I - D X;  X' = X Z2;  X'^T = Z2^T X^T
                for it in range(cfg.newton[j]):
                    last = (it == cfg.newton[j] - 1)
                    psz = psF.tile([P, 2 * P], F32, tag="mm")
                    nc.tensor.matmul(psz[:, 0:P], dT[:], xy[:, 0:P],
                                     start=True, stop=True)
                    z2 = stg.tile([P, P], BF16, tag="z2f", bufs=2)
                    nc.vector.tensor_tensor(z2[:], iden2[:], psz[:, 0:P],
                                            op=sub)
                    psp = psF.tile([P, 2 * P], F32, tag="mm")
                    nc.tensor.matmul(psp[:, 0:P], xy[:, P:2 * P], z2[:],
                                     start=True, stop=True)
                    nc.tensor.matmul(psp[:, P:2 * P], z2[:], xy[:, P:2 * P],
                                     start=True, stop=True)
                    xy2 = w2[j] if last else nwt.tile([P, 2 * P], BF16,
                                                      tag="xy")
                    nc.vector.tensor_copy(xy2[:], psp[:])
                    xy = xy2

            def fwd_sub(j):
                """Forward substitution on the local F slice."""
                psf = psF.tile([P, 2 * P], F32, tag="mm")
                for k in range(j):
                    nc.tensor.matmul(psf[:, 0:cw], etile(j, k), fY[k][:],
                                     start=(k == 0), stop=(k == j - 1))
                nc.vector.tensor_tensor(fY[j][:], fY[j][:], psf[:, 0:cw],
                                        op=sub)

            def panels(j, i0, i1, fast=False):
                """etile(i,j) = W^T tmp^T for i in [i0, i1), in place.

                Pairs of rows share the W^T weight load and fuse to one
                256-wide matmul.  fast=True keeps the copy-backs on the
                vector engine (next-pivot path); otherwise they ride the
                scalar engine.
                """
                if i0 >= i1:
                    return
                cp = nc.vector.tensor_copy if fast else (
                    lambda o, i_: nc.scalar.activation(o, i_, CopyFn))
                i = i0
                while i < i1:
                    if i + 1 < i1:
                        pst1 = psF.tile([P, P], BF16, tag="tpp", bufs=2)
                        nc.tensor.transpose(pst1[:], etile(i, j), iden[:])
                        pst2 = psF.tile([P, P], BF16, tag="tpp", bufs=2)
                        nc.tensor.transpose(pst2[:], etile(i + 1, j), iden[:])
                        tp2 = stg.tile([P, 2 * P], BF16, tag="tp2", bufs=2)
                        cp(tp2[:, 0:P], pst1[:])
                        cp(tp2[:, P:2 * P], pst2[:])
                        psl = psF.tile([P, 2 * P], F32, tag="mm")
                        nc.tensor.matmul(psl[:], wN(j), tp2[:],
                                         start=True, stop=True)
                        cp(etile(i, j), psl[:, 0:P])
                        cp(etile(i + 1, j), psl[:, P:2 * P])
                        i += 2
                    else:
                        tpt = pe_transpose(etile(i, j))
                        psl = psF.tile([P, 2 * P], F32, tag="mm")
                        nc.tensor.matmul(psl[:, 0:P], wN(j), tpt[:],
                                         start=True, stop=True)
                        cp(etile(i, j), psl[:, 0:P])
                        i += 1

            def urow(k, s):
                """U-row k of strip s (k < 2s): es[s][k] -= sum_{k'<k} terms.

                es[s][k] was DMA-preloaded with the gathered S block.
                Emitted right after panels(k-1) so it fills Newton stalls.
                """
                if k == 0:
                    return
                pst = psF.tile([P, 2 * P], F32, tag="mm")
                for k2 in range(k):
                    nc.tensor.matmul(pst[:], etile(k, k2), es[s][k2][:],
                                     start=(k2 == 0), stop=(k2 == k - 1))
                nc.vector.tensor_tensor(es[s][k][:], es[s][k][:], pst[:],
                                        op=sub)

            def lrow(i, j, s):
                """Strip-entry update of row i >= j=2s (kmax = j terms)."""
                if j == 0:
                    return
                pst = psF.tile([P, 2 * P], F32, tag="mm")
                for k2 in range(j):
                    nc.tensor.matmul(pst[:], etile(i, k2), es[s][k2][:],
                                     start=(k2 == 0), stop=(k2 == j - 1))
                nc.vector.tensor_tensor(es[s][i][:], es[s][i][:], pst[:],
                                        op=sub)

            def odd_row(i, j):
                """Apply the k=j-1 Schur term to es[s][i] right half."""
                s = j // 2
                pst = psF.tile([P, 2 * P], F32, tag="mm")
                nc.tensor.matmul(pst[:, 0:P], etile(i, j - 1),
                                 es[s][j - 1][:, P:2 * P],
                                 start=True, stop=True)
                rh = es[s][i][:, P:2 * P]
                nc.vector.tensor_tensor(rh, rh, pst[:, 0:P], op=sub)

            def transpose_inplace(blk):
                pst = psF.tile([P, P], BF16, tag="tpp", bufs=2)
                nc.tensor.transpose(pst[:], blk, iden[:])
                nc.scalar.activation(blk, pst[:], CopyFn)

            def fact_gen():
                """Factorization emission on a (column x quarter) frontier.

                Yields q before work that needs AllGather quarter q.  Bulk
                row work beyond the arrived-row frontier is deferred into
                per-quarter thunk lists, flushed when that quarter's yield
                passes.  fwd_subs go last (they need the F-pass fY tiles,
                which are emitted after the S-pass)."""
                deferred = [[], [], [], []]
                cur_q = 0
                yield 0

                def rows_split(i0, i1, fn):
                    """fn(a, b) inline for rows < frontier, else deferred."""
                    hi = (cur_q + 1) * QB
                    lo = min(max(i0, 0), i1)
                    if lo < min(i1, hi):
                        fn(lo, min(i1, hi))
                    r = max(lo, hi)
                    while r < i1:
                        q = r // QB
                        top = min(i1, (q + 1) * QB)
                        deferred[q].append(
                            lambda a=r, b=top, f=fn: f(a, b))
                        r = top

                for j in range(nb):
                    s, par = j // 2, j % 2
                    # pivot rows: j and j+1 for even j (strip entry),
                    # just j for odd j
                    qneed = (j + 1) // QB if par == 0 else j // QB
                    while cur_q < qneed:
                        cur_q += 1
                        yield cur_q
                        for th in deferred[cur_q]:
                            th()
                        deferred[cur_q] = []
                    if par == 0:
                        if j > 0:
                            lrow(j, j, s)
                            lrow(j + 1, j, s)
                        newton(j, etile(j, j))
                        rows_split(j + 2, nb,
                                   lambda a, b, jj=j, ss=s: [
                                       lrow(i, jj, ss) for i in range(a, b)])
                        # this strip's U blocks are final once all its
                        # lrows ran; transpose them for the back
                        # substitution after the last deferred reader
                        def utrans(jj=j, ss=s):
                            for i in range(jj):
                                for kk in range(2):
                                    transpose_inplace(
                                        es[ss][i][:, kk * P:(kk + 1) * P])
                        if j > 0:
                            if cur_q == 3:
                                utrans()
                            else:
                                deferred[3].append(utrans)
                        # panels: next-pivot pair on the fast path
                        rows_split(j + 1, min(j + 3, nb),
                                   lambda a, b, jj=j: panels(jj, a, b,
                                                             fast=True))
                        rows_split(j + 3, nb,
                                   lambda a, b, jj=j: panels(jj, a, b))
                    else:
                        odd_row(j, j)
                        newton(j, etile(j, j))
                        rows_split(j + 1, nb,
                                   lambda a, b, jj=j: [
                                       odd_row(i, jj) for i in range(a, b)])
                        # superdiagonal U(j-1, j): transpose after the last
                        # (possibly deferred) odd_row consumed it
                        def strans(jj=j, ss=s):
                            transpose_inplace(es[ss][jj - 1][:, P:2 * P])
                        if cur_q == 3:
                            strans()
                        else:
                            deferred[3].append(strans)
                        rows_split(j + 1, min(j + 3, nb),
                                   lambda a, b, jj=j: panels(jj, a, b,
                                                             fast=True))
                        rows_split(j + 3, nb,
                                   lambda a, b, jj=j: panels(jj, a, b))
                    # U-rows unlocked by panels(j): row j+1 of strips
                    # with 2s' > j+1 (defer if row j+1 hasn't arrived)
                    if j + 1 < nb:
                        if j + 1 < (cur_q + 1) * QB:
                            for s2 in range((j + 3) // 2, ns):
                                urow(j + 1, s2)
                        else:
                            deferred[(j + 1) // QB].append(
                                lambda k=j + 1: [urow(k, s2) for s2 in
                                                 range((k + 2) // 2, ns)])

                while cur_q < 3:
                    cur_q += 1
                    yield cur_q
                    for th in deferred[cur_q]:
                        th()
                    deferred[cur_q] = []
                # forward substitutions (need fY from the F-pass; emitted
                # last, the scheduler slots them where deps allow)
                yield 4
                for j in range(1, nb):
                    fwd_sub(j)

            gen = fact_gen()
            blocked_q = [None]
            gen_done = [False]

            def pump(max_q):
                if gen_done[0]:
                    return
                if blocked_q[0] is not None:
                    if blocked_q[0] > max_q:
                        return
                    blocked_q[0] = None
                while True:
                    try:
                        req = next(gen)
                    except StopIteration:
                        gen_done[0] = True
                        return
                    if req is not None and req > max_q:
                        blocked_q[0] = req
                        return

            # ---------------- Phase A: Gram ----------------
            with (
                tc.tile_pool(name="gram", bufs=1) as gram,
                tc.tile_pool(name="slabs", bufs=4) as slabs,
                tc.tile_pool(name="gsm", bufs=4) as gsm,
                tc.tile_pool(name="psA", bufs=2, space="PSUM") as psA,
            ):
                # Dummy collective to absorb the entry barrier early.
                nc.gpsimd.collective_compute(
                    "AllGather", mybir.AluOpType.bypass,
                    ins=[dummy_b.opt()], outs=[dummy_g.opt()],
                    replica_groups=rg)

                # PE warm-up while the first input DMAs stream.
                ps_w = psA.tile([P, 2 * cw], F32, tag="gps")
                for w in range(cfg.warm):
                    nc.tensor.matmul(ps_w[:, 0:P], iden[:], iden[:],
                                     start=(w == 0), stop=(w == cfg.warm - 1))
                w_sb = gsm.tile([1, 1], F32, tag="wsb")
                nc.vector.tensor_copy(w_sb[:], ps_w[0:1, 0:1])
                nc.sync.dma_start(sink[:], w_sb[:])

                # rhs in 4 chunks so the first matmul starts early
                RC = kt // 4  # k-tiles per rhs chunk
                rhs_c = [gram.tile([P, RC * 2 * cw], BF16, tag=f"rhs{c}",
                                   name=f"rhs{c}")
                         for c in range(4)]
                for c in range(4):
                    nc.sync.dma_start(
                        rhs_c[c][:],
                        rhs_in[:, c * RC * 2 * cw:(c + 1) * RC * 2 * cw])

                def rhs_sl(t, lo, hi):
                    c, tl = t // RC, t % RC
                    return rhs_c[c][:, tl * 2 * cw + lo:tl * 2 * cw + hi]

                b_sb = gram.tile([P, nb * v], BF16, tag="b_sb")
                nc.sync.dma_start(b_sb[:], b_in[:])
                sk_sb = gram.tile([P, nb * cw], BF16, tag="sk_sb")
                nc.sync.dma_start(sk_sb[:], sk_in[:])

                m22 = gram.tile([P, nb * cw], BF16, tag="m22")
                s_t = gram.tile([P, nb * cw], BF16, tag="s_t")

                # ---- S-pass: S row-blocks only, AG quarters kick early ----
                for m in range(nb):
                    slab1 = slabs.tile([P, kt * P], BF16, tag="slab")
                    nc.sync.dma_start(slab1[:],
                                      lt_in[(nb + m) * P:(nb + m + 1) * P, :])
                    slab0 = slabs.tile([P, kt * P], BF16, tag="slab")
                    nc.sync.dma_start(slab0[:], lt_in[m * P:(m + 1) * P, :])

                    msl = slice(m * cw, (m + 1) * cw)
                    # chain A: 0.5*M22 block-row (slab1 x rhs-h1 half)
                    ps = psA.tile([P, 2 * cw], F32, tag="gps")
                    for t in range(kt):
                        nc.tensor.matmul(ps[:, 0:cw],
                                         slab1[:, t * P:(t + 1) * P],
                                         rhs_sl(t, cw, 2 * cw),
                                         start=(t == 0), stop=(t == kt - 1))
                    nc.vector.tensor_copy(m22[:, msl], ps[:, 0:cw])
                    # chain B: 0.5*M11 block-row (slab0 x rhs-h0 half)
                    ps2 = psA.tile([P, 2 * cw], F32, tag="gps")
                    for t in range(kt):
                        nc.tensor.matmul(ps2[:, 0:cw],
                                         slab0[:, t * P:(t + 1) * P],
                                         rhs_sl(t, 0, cw),
                                         start=(t == 0), stop=(t == kt - 1))
                    t1 = gsm.tile([P, cw], F32, tag="t1")
                    nc.vector.tensor_tensor(t1[:], ps2[:, 0:cw], m22[:, msl],
                                            op=add)
                    nc.vector.tensor_tensor(s_t[:, msl], t1[:], sk_sb[:, msl],
                                            op=add)

                    if m % QB == QB - 1:
                        q = m // QB
                        nc.gpsimd.dma_start(
                            sag_b[q].rearrange("(t p) n -> p t n", p=P),
                            s_t[:, q * QB * cw:(q + 1) * QB * cw]
                            .rearrange("p (t n) -> p t n", n=cw))
                        nc.gpsimd.collective_compute(
                            "AllGather", mybir.AluOpType.bypass,
                            ins=[sag_b[q].opt()], outs=[sagg[q].opt()],
                            replica_groups=rg)
                        # DMA the gathered quarter straight into es tiles
                        for s in range(ns):
                            for t in range(QB):
                                r0 = (s * QB + t) * P
                                nc.sync.dma_start(
                                    es[s][q * QB + t][:],
                                    sagg[q][r0:r0 + P, :])

                    # interleave early factorization into the Gram stream
                    if m == 7:
                        pump(0)
                    elif m == 10:
                        pump(1)
                    elif m == 13:
                        pump(2)

                # ---- F-pass: F = M21 row-blocks (Newton-chain filler) ----
                for m in range(nb):
                    slab1 = slabs.tile([P, kt * P], BF16, tag="slab")
                    nc.sync.dma_start(slab1[:],
                                      lt_in[(nb + m) * P:(nb + m + 1) * P, :])
                    ps = psA.tile([P, 2 * cw], F32, tag="gps")
                    for t in range(kt):
                        nc.tensor.matmul(ps[:, 0:cw],
                                         slab1[:, t * P:(t + 1) * P],
                                         rhs_sl(t, 0, cw),
                                         start=(t == 0), stop=(t == kt - 1))
                    nc.vector.tensor_copy(fY[m][:], ps[:, 0:cw])

                # U1_c = (P @ B)[c-rows] : lhsT = m22 column slices (holds
                # 0.5*M22; b_sb holds 2*B, so the product is M22 @ B).
                u1s = gram.tile([P, (cw // P) * v], BF16, tag="u1s")
                for mh in range(cw // P):
                    psu = psA.tile([P, 2 * cw], F32, tag="gps")
                    for k in range(nb):
                        nc.tensor.matmul(
                            psu[:, 0:v],
                            m22[:, k * cw + mh * P:k * cw + (mh + 1) * P],
                            b_sb[:, k * v:(k + 1) * v],
                            start=(k == 0), stop=(k == nb - 1))
                    nc.vector.tensor_copy(u1s[:, mh * v:(mh + 1) * v],
                                          psu[:, 0:v])
                nc.gpsimd.dma_start(
                    u1_b[:].rearrange("(t p) n -> p t n", p=P),
                    u1s[:].rearrange("p (t n) -> p t n", n=v))
                nc.gpsimd.collective_compute(
                    "AllGather", mybir.AluOpType.bypass,
                    ins=[u1_b.opt()], outs=[u1ag.opt()], replica_groups=rg)

                pump(99)  # rest of the factorization + fwd_subs
                assert gen_done[0]

            # ------- back substitution + output chain -------
            with (
                tc.tile_pool(name="chain", bufs=1) as chain,
                tc.tile_pool(name="stg2", bufs=2) as stg2,
                tc.tile_pool(name="psC", bufs=1, space="PSUM") as psC,
            ):
                u1_sb = chain.tile([P, nb * v], BF16, tag="u1_sb")
                nc.sync.dma_start(
                    u1_sb[:].rearrange("p (t n) -> p t n", n=v),
                    u1ag[:, :].rearrange("(t p) n -> p t n", p=P))
                bt_sb = chain.tile([P, nv * d], BF16, tag="bt_sb")
                nc.sync.dma_start(bt_sb[:], bt_in[:])
                # fp32 copies of the back-sub results for the final output
                xF = [chain.tile([P, cw], F32, tag=f"xF{j}", name=f"xF{j}")
                      for j in range(nb)]
                t2ps = [psC.tile([P, cw], F32, tag=f"t2ps{vh}",
                                 name=f"t2ps{vh}")
                        for vh in range(nv)]
                # back substitution (X overwrites fY); etile(j, k) for
                # k > j already holds U^T from the eager transposes.
                for j in range(nb - 1, -1, -1):
                    if j < nb - 1:
                        psz = psF.tile([P, 2 * P], F32, tag="mm")
                        for k in range(j + 1, nb):
                            nc.tensor.matmul(psz[:, 0:cw], etile(j, k),
                                             fY[k][:],
                                             start=(k == j + 1),
                                             stop=(k == nb - 1))
                        z = stg2.tile([P, cw], BF16, tag="z")
                        nc.vector.tensor_tensor(z[:], fY[j][:], psz[:, 0:cw],
                                                op=sub)
                    else:
                        z = fY[j]
                    psx = psF.tile([P, 2 * P], F32, tag="mm")
                    nc.tensor.matmul(psx[:, 0:cw], wT(j), z[:],
                                     start=True, stop=True)
                    nc.vector.tensor_copy(fY[j][:], psx[:, 0:cw])
                    nc.scalar.activation(xF[j][:], psx[:, 0:cw], CopyFn)
                    for vh in range(nv):
                        nc.tensor.matmul(
                            t2ps[vh][:],
                            u1_sb[:, j * v + vh * P:j * v + (vh + 1) * P],
                            fY[j][:], start=(j == nb - 1),
                            stop=(j == 0))
                t2 = [chain.tile([P, cw], BF16, tag=f"t2_{vh}",
                                 name=f"t2_{vh}") for vh in range(nv)]
                for vh in range(nv):
                    nc.vector.tensor_copy(t2[vh][:], t2ps[vh][:])
                for m in range(nb):
                    ps3 = psF.tile([P, 2 * P], F32, tag="mm")
                    for vh in range(nv):
                        nc.tensor.matmul(
                            ps3[:, 0:cw],
                            bt_sb[:, vh * d + m * P:vh * d + (m + 1) * P],
                            t2[vh][:], start=(vh == 0),
                            stop=(vh == nv - 1))
                    ao = chain.tile([P, cw], F32, tag="ao", bufs=2)
                    nc.vector.tensor_tensor(ao[:], xF[m][:], ps3[:, 0:cw],
                                            op=sub)
                    nc.sync.dma_start(a_out[:, m * cw:(m + 1) * cw],
                                      ao[:])

                # Anchor the entry-barrier dummy AllGather against DCE
                # (emitted last so its sagg-read never stalls a real DMA).
                dmy = chain.tile([1, 2], BF16, tag="dmy")
                nc.sync.dma_start(dmy[:], dummy_g[0:1, :])
                nc.sync.dma_start(sink2[:], dmy[:])

    nc.compile()
    return nc


_CACHE = {}


def _get_program(cfg: Cfg):
    key = (cfg.d, cfg.v, cfg.ncores, tuple(cfg.newton), cfg.warm)
    if key not in _CACHE:
        _CACHE[key] = build_program(cfg)
    return _CACHE[key]


def run(cfg: Cfg, L, R, B, trace=False):
    global LAST_EXEC_NS
    d, cw, v, nb, kt = cfg.d, cfg.cw, cfg.v, cfg.nb, cfg.kt
    nc = _get_program(cfg)
    L = np.ascontiguousarray(L, np.float32)
    R = np.ascontiguousarray(R, np.float32)
    B = np.ascontiguousarray(B, np.float32)
    LT = np.ascontiguousarray(L.T)

    # lt_t[m, p, t, q] = LT[t*128+p, m*128+q]; h=0 tiles (m < nb) * 0.5
    Y = LT.reshape(kt, P, kt, P)
    lt_t = np.ascontiguousarray(Y.transpose(2, 1, 0, 3))
    lt_t[:nb] *= 0.5
    lt_t = lt_t.reshape(cfg.kdim, cfg.kdim).astype(ml_dtypes.bfloat16)

    SK = 0.5 * (R - R.T)
    b2 = (2.0 * B).reshape(nb, P, v).transpose(1, 0, 2) \
        .reshape(P, nb * v).astype(ml_dtypes.bfloat16)
    bt_t = np.ascontiguousarray(B.T).reshape(cfg.nv, P, d) \
        .transpose(1, 0, 2).reshape(P, cfg.nv * d).astype(ml_dtypes.bfloat16)
    iden = np.eye(P, dtype=np.float32).astype(ml_dtypes.bfloat16)
    iden2 = 2.0 * np.eye(P, dtype=np.float32)
    ones_v = np.ones((P, 1), np.float32)
    ones_r = np.ones((1, P), np.float32)

    in_maps = []
    for c in range(cfg.ncores):
        c0 = c * cw
        rhs1 = LT[:, c0:c0 + cw].reshape(kt, P, cw).transpose(1, 0, 2)
        rhs2 = 0.5 * LT[:, d + c0:d + c0 + cw].reshape(kt, P, cw) \
            .transpose(1, 0, 2)
        rhs = np.concatenate([rhs1, rhs2], axis=2) \
            .reshape(P, kt * 2 * cw).astype(ml_dtypes.bfloat16)
        sk_c = np.ascontiguousarray(
            SK[:, c0:c0 + cw].reshape(nb, P, cw).transpose(1, 0, 2)
            .reshape(P, nb * cw)).astype(ml_dtypes.bfloat16)
        in_maps.append({
            "lt": lt_t,
            "rhs": rhs,
            "b": b2, "bt": bt_t, "sk": sk_c,
            "iden": iden, "iden2": iden2,
            "onesv": ones_v, "onesr": ones_r,
        })
    res = run_bass_kernel_spmd(nc, in_maps, core_ids=list(range(cfg.ncores)),
                               trace=trace)
    LAST_EXEC_NS = res.exec_time_ns
    run.last_results = res.results
    cols = []
    for c in range(cfg.ncores):
        a_t = res.results[c]["a_out"]  # [128, nb*cw]
        cols.append(a_t.reshape(P, nb, cw).transpose(1, 0, 2).reshape(d, cw))
    A = np.concatenate(cols, axis=1)
    return np.ascontiguousarray(A, np.float32)


def kernel(L, R, B, dim_K):
    dim = int(dim_K)
    assert dim == 2048 and L.shape == (4096, 4096)
    cfg = Cfg(d=2048, v=256, ncores=8)
    return run(cfg, L, R, B, trace=False)
